# revision 1
# baseline (speedup 1.0000x reference)
"""Trainium2 Bass kernel for nn_LSTM2: 512-step LSTM+MLP recurrence, TP-8.

Sharding: tensor-parallel over hidden dim C=1024 (each of 8 cores owns a
128-channel block of c/h and the matching i/f/g/o gate rows); batch B=128 sits
on PSUM partitions.  Per step, two 64KB/core ncfw AllGathers (HBM bounce)
share h_lstm^T and h1r^T across the chip.
Fusion: W2h = w_hh @ w2 folds the second MLP matmul into the gate matmul;
final projection uses Wo2 = wo @ w2.  Matmuls run in float32r (FP22 multiply,
FP32 accumulate, 1.5 cyc/row).
"""

import sys

sys.path.insert(0, "/opt/trn_rl_repo")

import numpy as np

import concourse.bass as bass
import concourse.mybir as mybir
from concourse import bass_utils

T1, T2, B, D = 8, 64, 128, 512
C, O = 1024, 3
T = T1 * T2  # 512
NSH = 8
CK = C // 128  # 8
DK = D // 128  # 4
GW = 4 * 128  # gate cols per core

F32R = mybir.dt.float32r
F32 = mybir.dt.float32


def _host_prep(x, w_ih, b_ih, w_hh, b_hh, w1, b1, w2, b2, wo, bo):
    x = np.asarray(x, np.float32)
    w_ih = np.asarray(w_ih, np.float32)
    w_hh = np.asarray(w_hh, np.float32)
    w1 = np.asarray(w1, np.float32)
    w2 = np.asarray(w2, np.float32)
    wo = np.asarray(wo, np.float32)
    b_tot_full = (
        np.asarray(b_ih, np.float32)
        + np.asarray(b_hh, np.float32)
        + w_hh @ np.asarray(b2, np.float32)
    )
    W2h = w_hh @ w2
    Wo2 = wo @ w2
    b_o2 = wo @ np.asarray(b2, np.float32) + np.asarray(bo, np.float32)
    b1 = np.asarray(b1, np.float32)

    xf = x.reshape(T, B, D)
    xp = np.ascontiguousarray(
        xf.reshape(T, B, DK, 128).transpose(0, 3, 2, 1).reshape(T, 128, DK * B)
    )
    eye = np.ascontiguousarray(np.eye(128, dtype=np.float32))
    ones_row = np.ones((1, B), np.float32)

    in_maps = []
    for k in range(NSH):
        gc = np.concatenate(
            [np.arange(g * C + k * 128, g * C + (k + 1) * 128) for g in range(4)]
        )
        wih_k = w_ih[gc]  # (GW, D)
        wih_pack = np.ascontiguousarray(
            wih_k.T.reshape(DK, 128, GW).transpose(1, 0, 2).reshape(128, DK * GW)
        )
        w2h_k = W2h[gc]  # (GW, C)
        w2h_pack = np.empty((128, CK * GW), np.float32)
        w1_k = w1[k * 128 : (k + 1) * 128]  # (128, C)
        w1_pack = np.empty((128, CK * 128), np.float32)
        wo2_pack = np.empty((128, CK * O), np.float32)
        for j in range(CK):  # slot j = chunk j (rank order, ncfw AllGather)
            w2h_pack[:, j * GW : (j + 1) * GW] = w2h_k[:, j * 128 : (j + 1) * 128].T
            w1_pack[:, j * 128 : (j + 1) * 128] = w1_k[:, j * 128 : (j + 1) * 128].T
            wo2_pack[:, j * O : (j + 1) * O] = Wo2[:, j * 128 : (j + 1) * 128].T
        in_maps.append(
            {
                "x_perm": xp,
                "wih_pack": wih_pack,
                "w2h_pack": np.ascontiguousarray(w2h_pack),
                "w1_pack": np.ascontiguousarray(w1_pack),
                "wo2_pack": np.ascontiguousarray(wo2_pack),
                "btot_row": np.ascontiguousarray(b_tot_full[gc][None, :]),
                "b1_col": np.ascontiguousarray(b1[k * 128 : (k + 1) * 128][:, None]),
                "bo2_row": np.ascontiguousarray(b_o2[None, :]),
                "ones_row": ones_row,
                "ident": eye,
            }
        )
    return in_maps


def build_kernel(n_steps=T):
    from contextlib import ExitStack

    nc = bass.Bass(debug=False, target_bir_lowering=False)
    RG = [list(range(NSH))]

    xd = nc.dram_tensor("x_perm", [n_steps, 128, DK * B], F32R, kind="ExternalInput")
    wihd = nc.dram_tensor("wih_pack", [128, DK * GW], F32R, kind="ExternalInput")
    w2hd = nc.dram_tensor("w2h_pack", [128, CK * GW], F32R, kind="ExternalInput")
    w1d = nc.dram_tensor("w1_pack", [128, CK * 128], F32R, kind="ExternalInput")
    wo2d = nc.dram_tensor("wo2_pack", [128, CK * O], F32R, kind="ExternalInput")
    btotd = nc.dram_tensor("btot_row", [1, GW], F32R, kind="ExternalInput")
    b1d = nc.dram_tensor("b1_col", [128, 1], F32, kind="ExternalInput")
    bo2d = nc.dram_tensor("bo2_row", [1, O], F32R, kind="ExternalInput")
    onesd = nc.dram_tensor("ones_row", [1, B], F32R, kind="ExternalInput")
    identd = nc.dram_tensor("ident", [128, 128], F32R, kind="ExternalInput")
    outd = nc.dram_tensor("out", [O, B], F32, kind="ExternalOutput")

    # collective bounce / gather DRAM (double-buffered)
    bhl_d = [nc.dram_tensor(f"bhl{i}", [128, 128], F32R) for i in range(2)]
    bh1_d = [nc.dram_tensor(f"bh1{i}", [128, 128], F32R) for i in range(2)]
    ghl_d = [
        nc.dram_tensor(f"ghl{i}", [NSH * 128, 128], F32R, addr_space="Shared")
        for i in range(2)
    ]
    gh1_d = [
        nc.dram_tensor(f"gh1{i}", [NSH * 128, 128], F32R, addr_space="Shared")
        for i in range(2)
    ]

    ctx = ExitStack()
    sb = lambda name, shape, dt=F32R: ctx.enter_context(nc.sbuf_tensor(name, shape, dt))
    ps = lambda name, shape, dt=F32: ctx.enter_context(nc.psum_tensor(name, shape, dt))
    sem = lambda name: ctx.enter_context(nc.semaphore(name))

    with ctx:
        wih_sb = sb("wih_sb", [128, DK * GW])
        w2h_sb = sb("w2h_sb", [128, CK * GW])
        w1_sb = sb("w1_sb", [128, CK * 128])
        wo2_sb = sb("wo2_sb", [128, CK * O])
        btot_sb = sb("btot_sb", [1, GW])
        b1_sb = sb("b1_sb", [128, 1], F32)
        bo2_sb = sb("bo2_sb", [1, O])
        ones_sb = sb("ones_sb", [1, B])
        ident_sb = sb("ident_sb", [128, 128])
        x_sb = [sb(f"x_sb{i}", [128, DK * B]) for i in range(3)]
        h1g = [sb(f"h1g{i}", [128, CK * 128]) for i in range(2)]
        hlg = [sb(f"hlg{i}", [128, CK * 128]) for i in range(2)]
        hl_loc = [sb(f"hl_loc{i}", [128, 128]) for i in range(2)]
        h1_loc = [sb(f"h1_loc{i}", [128, 128]) for i in range(2)]
        c_sb = [sb(f"c_sb{i}", [128, 128], F32) for i in range(2)]
        hl_sb = [sb(f"hl_sb{i}", [128, 128]) for i in range(2)]
        sig_i = [sb(f"sig_i{i}", [128, 128], F32) for i in range(2)]
        sig_f = [sb(f"sig_f{i}", [128, 128], F32) for i in range(2)]
        sig_o = [sb(f"sig_o{i}", [128, 128], F32) for i in range(2)]
        tanh_g = [sb(f"tanh_g{i}", [128, 128], F32) for i in range(2)]
        tanh_c = [sb(f"tanh_c{i}", [128, 128], F32) for i in range(2)]
        t1_sb = sb("t1_sb", [128, 128], F32)
        out_sb = sb("out_sb", [O, B], F32)

        psum_g = [ps(f"psum_g{i}", [128, GW]) for i in range(2)]
        psum_tr = [ps(f"psum_tr{i}", [128, 128], F32R) for i in range(2)]
        psum_m = [ps(f"psum_m{i}", [128, 128]) for i in range(2)]
        psum_o = ps("psum_o", [O, B])

        s_init = sem("s_init")
        s_x = sem("s_x")
        s_xfree = sem("s_xfree")
        s_gates = sem("s_gates")
        s_act_ig = sem("s_act_ig")
        s_act_f = sem("s_act_f")
        s_act_o = sem("s_act_o")
        s_c = sem("s_c")
        s_cdone = sem("s_cdone")
        s_elem = sem("s_elem")
        s_trans = sem("s_trans")
        s_tcopy = sem("s_tcopy")
        s_mlp = sem("s_mlp")
        s_relu = sem("s_relu")
        s_bhl = sem("s_bhl")
        s_cchl = sem("s_cchl")
        s_ghl = sem("s_ghl")
        s_bh1 = sem("s_bh1")
        s_cch1 = sem("s_cch1")
        s_gh1 = sem("s_gh1")
        s_out = sem("s_out")

        N_INIT_DMA = 11  # 9 consts + x(0) + x(1)
        INIT = 16 * N_INIT_DMA

        def gin_view(d):  # DRAM gather [8*128,128] -> [q,j,b] matching SBUF
            return d[:].rearrange("(j q) b -> q j b", q=128)

        with nc.Block() as block:

            @block.sync
            def _(sp):
                for src, dst in [
                    (wihd, wih_sb),
                    (w2hd, w2h_sb),
                    (w1d, w1_sb),
                    (wo2d, wo2_sb),
                    (btotd, btot_sb),
                    (b1d, b1_sb),
                    (bo2d, bo2_sb),
                    (onesd, ones_sb),
                    (identd, ident_sb),
                ]:
                    sp.dma_start(dst[:], src[:]).then_inc(s_init, 16)
                sp.dma_start(x_sb[0][:], xd[0]).then_inc(s_init, 16)
                if n_steps > 1:
                    sp.dma_start(x_sb[1][:], xd[1]).then_inc(s_init, 16)
                for t in range(n_steps):
                    p = t % 2
                    if t + 2 < n_steps:
                        if t >= 2:
                            sp.wait_ge(s_xfree, t - 1)
                        sp.dma_start(x_sb[(t + 2) % 3][:], xd[t + 2]).then_inc(s_x, 16)
                    # h_lstm^T chunk out to bounce, gathered back after AG
                    if t >= 2:
                        sp.wait_ge(s_cchl, t - 1)
                    sp.wait_ge(s_tcopy, t + 1)
                    sp.dma_start(bhl_d[p][:], hl_loc[p][:]).then_inc(s_bhl, 16)
                    sp.wait_ge(s_cchl, t + 1)
                    sp.dma_start(
                        hlg[p][:].rearrange("q (j b) -> q j b", b=128),
                        gin_view(ghl_d[p]),
                    ).then_inc(s_ghl, 16)
                    # h1r^T chunk
                    if t >= 2:
                        sp.wait_ge(s_cch1, t - 1)
                    sp.wait_ge(s_relu, t + 1)
                    sp.dma_start(bh1_d[p][:], h1_loc[p][:]).then_inc(s_bh1, 16)
                    sp.wait_ge(s_cch1, t + 1)
                    sp.dma_start(
                        h1g[p][:].rearrange("q (j b) -> q j b", b=128),
                        gin_view(gh1_d[p]),
                    ).then_inc(s_gh1, 16)
                # output writeback
                sp.wait_ge(s_out, 2)
                sp.dma_start(outd[:], out_sb[:]).then_inc(s_out, 16)
                sp.wait_ge(s_out, 18)

            @block.gpsimd
            def _(pool):
                for t in range(n_steps):
                    p = t % 2
                    pool.wait_ge(s_bhl, 16 * (t + 1))
                    if t >= 2:
                        pool.wait_ge(s_ghl, 16 * (t - 1))
                    pool.collective_compute(
                        "AllGather",
                        mybir.AluOpType.bypass,
                        replica_groups=RG,
                        ins=[bhl_d[p][:]],
                        outs=[ghl_d[p][:]],
                    ).then_inc(s_cchl, 1)
                    pool.wait_ge(s_bh1, 16 * (t + 1))
                    if t >= 2:
                        pool.wait_ge(s_gh1, 16 * (t - 1))
                    pool.collective_compute(
                        "AllGather",
                        mybir.AluOpType.bypass,
                        replica_groups=RG,
                        ins=[bh1_d[p][:]],
                        outs=[gh1_d[p][:]],
                    ).then_inc(s_cch1, 1)

            @block.tensor
            def _(pe):
                def bias_x_mms(t, close):
                    g = psum_g[t % 2]
                    pe.matmul(
                        g[:], ones_sb[:], btot_sb[:],
                        start=True, stop=False, skip_group_check=True,
                    )
                    mm = None
                    for d in range(DK):
                        mm = pe.matmul(
                            g[:],
                            x_sb[t % 3][:, d * B : (d + 1) * B],
                            wih_sb[:, d * GW : (d + 1) * GW],
                            start=False,
                            stop=close and (d == DK - 1),
                            skip_group_check=True,
                        )
                    return mm

                pe.wait_ge(s_init, INIT)
                for t in range(n_steps):
                    g = psum_g[t % 2]
                    if t == 0:
                        bias_x_mms(0, close=True).then_inc(s_gates, 1)
                    else:
                        pe.wait_ge(s_gh1, 16 * t)  # gathered h1r(t-1)
                        hbuf = h1g[(t - 1) % 2]
                        for j in range(CK):
                            mm = pe.matmul(
                                g[:],
                                hbuf[:, j * 128 : (j + 1) * 128],
                                w2h_sb[:, j * GW : (j + 1) * GW],
                                start=False,
                                stop=(j == CK - 1),
                                skip_group_check=True,
                            )
                        mm.then_inc(s_gates, 1)

                    if t + 1 < n_steps:
                        if t >= 1:
                            pe.wait_ge(s_x, 16 * t)  # x(t+1) loaded
                            pe.wait_ge(s_act_o, t)  # psum parity consumed
                        bias_x_mms(t + 1, close=False).then_inc(s_xfree, 1)

                    pe.wait_ge(s_elem, t + 1)
                    if t >= 2:
                        pe.wait_ge(s_tcopy, t - 1)
                    pe.matmul(
                        psum_tr[t % 2][:], hl_sb[t % 2][:], ident_sb[:],
                        is_transpose=True,
                    ).then_inc(s_trans, 1)

                    pe.wait_ge(s_ghl, 16 * (t + 1))  # gathered h_lstm(t)
                    m = psum_m[t % 2]
                    for j in range(CK):
                        mm = pe.matmul(
                            m[:],
                            w1_sb[:, j * 128 : (j + 1) * 128],
                            hlg[t % 2][:, j * 128 : (j + 1) * 128],
                            start=(j == 0),
                            stop=(j == CK - 1),
                            skip_group_check=True,
                        )
                    mm.then_inc(s_mlp, 1)

                tl = (n_steps - 1) % 2
                pe.wait_ge(s_gh1, 16 * n_steps)
                pe.matmul(
                    psum_o[:], bo2_sb[:], ones_sb[:],
                    start=True, stop=False, skip_group_check=True,
                )
                for j in range(CK):
                    mm = pe.matmul(
                        psum_o[:],
                        wo2_sb[:, j * O : (j + 1) * O],
                        h1g[tl][:, j * 128 : (j + 1) * 128],
                        start=False, stop=(j == CK - 1), skip_group_check=True,
                    )
                mm.then_inc(s_out, 1)

            @block.scalar
            def _(act):
                AF = mybir.ActivationFunctionType
                act.wait_ge(s_init, INIT)
                for t in range(n_steps):
                    p = t % 2
                    g = psum_g[p]
                    act.wait_ge(s_gates, t + 1)
                    if t >= 2:
                        act.wait_ge(s_elem, t - 1)
                    act.activation(sig_i[p][:], g[:, 0:128], AF.Sigmoid)
                    act.activation(tanh_g[p][:], g[:, 256:384], AF.Tanh).then_inc(
                        s_act_ig, 1
                    )
                    act.activation(sig_f[p][:], g[:, 128:256], AF.Sigmoid).then_inc(
                        s_act_f, 1
                    )
                    act.activation(sig_o[p][:], g[:, 384:512], AF.Sigmoid).then_inc(
                        s_act_o, 1
                    )
                    act.wait_ge(s_c, t + 1)
                    act.activation(tanh_c[p][:], c_sb[p][:], AF.Tanh).then_inc(
                        s_cdone, 1
                    )
                    act.wait_ge(s_mlp, t + 1)
                    if t >= 2:
                        act.wait_ge(s_bh1, 16 * (t - 1))  # h1_loc parity free
                    act.activation(
                        h1_loc[p][:], psum_m[p][:], AF.Relu, bias=b1_sb[:]
                    ).then_inc(s_relu, 1)

            @block.vector
            def _(dve):
                dve.wait_ge(s_init, INIT)
                for t in range(n_steps):
                    p = t % 2
                    dve.wait_ge(s_act_ig, t + 1)
                    if t == 0:
                        dve.tensor_mul(c_sb[0][:], sig_i[0][:], tanh_g[0][:]).then_inc(
                            s_c, 1
                        )
                    else:
                        dve.tensor_mul(t1_sb[:], sig_i[p][:], tanh_g[p][:])
                        dve.wait_ge(s_act_f, t + 1)
                        if t >= 2:
                            dve.wait_ge(s_cdone, t - 1)
                        dve.tensor_mul(c_sb[p][:], sig_f[p][:], c_sb[1 - p][:])
                        dve.tensor_add(c_sb[p][:], c_sb[p][:], t1_sb[:]).then_inc(
                            s_c, 1
                        )
                    dve.wait_ge(s_act_o, t + 1)
                    dve.wait_ge(s_cdone, t + 1)
                    if t >= 2:
                        dve.wait_ge(s_trans, t - 1)
                    dve.tensor_mul(hl_sb[p][:], sig_o[p][:], tanh_c[p][:]).then_inc(
                        s_elem, 1
                    )
                    dve.wait_ge(s_trans, t + 1)
                    if t >= 2:
                        dve.wait_ge(s_bhl, 16 * (t - 1))  # hl_loc parity free
                    dve.tensor_copy(hl_loc[p][:], psum_tr[p][:]).then_inc(s_tcopy, 1)
                dve.wait_ge(s_out, 1)
                dve.tensor_copy(out_sb[:], psum_o[:]).then_inc(s_out, 1)

    return nc


def kernel(**inputs):
    in_maps = _host_prep(**inputs)
    nc = build_kernel(n_steps=T)
    res = bass_utils.run_bass_kernel_spmd(nc, in_maps, core_ids=list(range(NSH)))
    out = res.results[0]["out"]
    return np.ascontiguousarray(np.asarray(out, np.float32).T)

